# revision 47
# baseline (speedup 1.0000x reference)
"""Trainium2 Bass kernel for nn_CrossAssetAttentionNetwork.

Sharding: data-parallel over batch — 8 batches -> 8 NeuronCores, full
[N,N] attention per core, small weights replicated.

Algebraic simplifications (inherited from the baseline):
 1. winner = sigmoid(attn @ (v @ Ws) + bs): the PV matmul collapses to
    an N-vector vw = x @ (Wv.T @ Ws) + bv.Ws (host-computed in f64).
 2. Sorting queries+keys by price_rank (host-side) makes the
    non-constant part of the gate a narrow static band: outside a
    384-wide window per 128-query block, gate == Gv19 (= gate of the
    clipped bucket), applied as the ACT *scale* input for free.

This version (~77us vs the 104us baseline):
 - fp8e4 (e4m3) everywhere on the PE with DoubleRow perf mode (two
   128-row k-tiles per instruction): projections AND the [N,N] score
   matmuls.  x ships as fp8 (1MB), gains folded host-side
   (x*2, W*8 -> scores*256, exp scale /256).
 - Single whole-row exp per block on ACT (scale=Gv19, accum -> Z);
   the 384-col band is gate-multiplied in PSUM on DVE first (band
   host-prescaled by 1/Gv19 so one scale serves the whole row).
 - W1 = sum E*vw as one full-row DVE STT per block (steady state is
   DVE-bound at ~2.7us/block: gate 0.57 + STT 2.29).  For the last
   two blocks POOL_COLS columns move to Pool (product) + ACT
   (copy-accumulate) to shorten the serial tail.
 - Minimal 384-wide gate band -> gband input is 1.57MB (was 2.9MB);
   vw ships pre-replicated to all 128 partitions (no on-chip bcast).
 - DMA: x as ONE 8KB-per-partition transfer first on the sync queue
   (large rows sustain ~280GB/s; small ones crawl), weights/consts on
   the scalar queue, gband streamed per-4-block-group on both.
 - Projections region-major with evacs AFTER all 8 matmuls of a tile
   (mid-tile evacs trip conservative subtile WAR serialization);
   qT ci2/3 computed before ci0/1 so block 0's S tile takes the
   earliest-freed PSUM slot; qA evacuates cols 0:256 first (all that
   blocks 0-1 need as lhsT).
 - finish (sigmoid) in two batched halves (after block 8 + at end).
"""

import numpy as np
from contextlib import ExitStack

import concourse.bass as bass
import concourse.mybir as mybir
import concourse.tile as tile
from concourse import bacc
from concourse.bass_utils import run_bass_kernel_spmd

B, N, DIN, DOUT = 8, 2048, 512, 256
NUM_BUCKETS = 20
P = 128
NBLK = N // P            # 16 query blocks
OC = DOUT // P           # 2 chunks of the head dim
NPAIR = DIN // (2 * P)   # 2 DoubleRow pair-chunks of the input dim
CCOL = 512               # score column tile
NCCOL = N // CCOL        # 4
GW = 384                 # gate band window width per block
WPAD = (GW - P) // 2
XG = 2.0                 # fp8 gain on x
WG = 8.0                 # fp8 gain on Wq/Wk ; scores scale = XG*WG = 16
SGAIN = XG * WG
POOL_COLS = 640          # W1 columns offloaded to Pool+ACT in the tail

F32 = mybir.dt.float32
BF16 = mybir.dt.bfloat16
FP8 = mybir.dt.float8e4

Act = mybir.ActivationFunctionType
Alu = mybir.AluOpType
DR = mybir.MatmulPerfMode.DoubleRow

LAST_EXEC_NS = None


def _win_start(b):
    return min(max(P * b - WPAD, 0), N - GW)


def _build(nc, bs_val: float):
    xp_in = nc.dram_tensor("xp", [P, 2 * NPAIR * N], FP8,
                           kind="ExternalInput").ap()
    wq_in = nc.dram_tensor("wq8", [P, 2 * NPAIR * DOUT], FP8,
                           kind="ExternalInput").ap()
    wk_in = nc.dram_tensor("wk8", [P, 2 * NPAIR * DOUT], FP8,
                           kind="ExternalInput").ap()
    bqk_in = nc.dram_tensor("bqk", [P, 2 * OC], F32, kind="ExternalInput").ap()
    gv19_in = nc.dram_tensor("gv19", [P, 1], F32, kind="ExternalInput").ap()
    vb_in = nc.dram_tensor("vb", [P, N], BF16, kind="ExternalInput").ap()
    gband_in = nc.dram_tensor("gband", [P, NBLK * GW], BF16,
                              kind="ExternalInput").ap()
    out = nc.dram_tensor("out", [P, NBLK], F32, kind="ExternalOutput").ap()

    with tile.TileContext(nc) as tc, ExitStack() as ctx:
        consts = ctx.enter_context(tc.tile_pool(name="consts", bufs=1))

        xp_sb = consts.tile([P, NPAIR, 2, N], FP8, tag="xp")
        wq_sb = consts.tile([P, NPAIR, 2, DOUT], FP8, tag="wq")
        wk_sb = consts.tile([P, NPAIR, 2, DOUT], FP8, tag="wk")
        bqk_sb = consts.tile([P, 2 * OC], F32, tag="bqk")
        gv19_sb = consts.tile([P, 1], F32, tag="gv19")
        qT_sb = consts.tile([P, OC, N], FP8, tag="qT")
        kT_sb = consts.tile([P, OC, N], FP8, tag="kT")
        gb_sb = consts.tile([P, NBLK * GW], BF16, tag="gb")
        vb_sb = consts.tile([P, N], BF16, tag="vb")
        nbs_sb = consts.tile([P, 1], F32, tag="nbs")
        zall_sb = consts.tile([P, NBLK], F32, tag="zall")
        w1d_sb = consts.tile([P, NBLK], F32, tag="w1d")
        w1p_sb = consts.tile([P, NBLK], F32, tag="w1p")
        wout_sb = consts.tile([P, NBLK], F32, tag="wout")
        nc.vector.memset(nbs_sb[:], -float(bs_val))

        # --- DMAs.  Per-queue delivery is strictly in program order.
        # sync queue: x first as ONE 8KB-per-partition transfer (large
        # rows sustain much higher DMA throughput), then wq and half
        # the gate band.  scalar queue: weights/consts, the replicated
        # vw rows, the other half of the band.
        x_flat = xp_sb[:].rearrange("p c t n -> p (c t n)")
        nc.sync.dma_start(x_flat[:], xp_in[:])
        nc.sync.dma_start(wq_sb[:].rearrange("p c t o -> p (c t o)"), wq_in)
        nc.scalar.dma_start(wk_sb[:].rearrange("p c t o -> p (c t o)"), wk_in)
        nc.scalar.dma_start(gv19_sb[:], gv19_in)
        nc.scalar.dma_start(bqk_sb[:], bqk_in)
        nc.scalar.dma_start(vb_sb[:], vb_in)
        GQ = 4 * GW
        for g in range(NBLK // 4):
            eng = nc.scalar if g % 2 == 0 else nc.sync
            eng.dma_start(gb_sb[:, g * GQ:(g + 1) * GQ],
                          gband_in[:, g * GQ:(g + 1) * GQ])

        psS = ctx.enter_context(tc.tile_pool(name="psS", bufs=2, space="PSUM"))
        epool = ctx.enter_context(tc.tile_pool(name="e", bufs=4))
        sd_pool = ctx.enter_context(tc.tile_pool(name="scrd", bufs=3))
        sp_pool = ctx.enter_context(tc.tile_pool(name="scrp", bufs=2))
        spool = ctx.enter_context(tc.tile_pool(name="small", bufs=3))

        def evac(dst, src, bias_col, eng):
            # PSUM f32 -> SBUF fp8 with per-partition bias
            if eng == "dve":
                nc.vector.tensor_scalar_add(dst, src, bqk_sb[:, bias_col:bias_col + 1])
            else:
                nc.scalar.activation(dst, src, Act.Identity,
                                     bias=bqk_sb[:, bias_col:bias_col + 1])

        def proj(pt, w_sb, t_sb, half, bias0, alt):
            # all 8 DoubleRow matmuls of the tile first, then the two
            # [128,1024] evacs (one DVE one ACT, in parallel) — evacs
            # emitted mid-tile create false write-after-read stalls.
            # pair-OUTER: consecutive matmuls hit different PSUM
            # regions, so they overlap like the score matmuls do
            # (back-to-back accumulating pairs serialize on the PE).
            for pair in range(NPAIR):
                for oc in range(OC):
                    for j in range(2):
                        ci = 2 * half + j
                        nc.tensor.matmul(
                            pt[:, oc, j * CCOL:(j + 1) * CCOL],
                            lhsT=w_sb[:, pair, :, oc * P:(oc + 1) * P],
                            rhs=xp_sb[:, pair, :, ci * CCOL:(ci + 1) * CCOL],
                            start=(pair == 0), stop=(pair == NPAIR - 1),
                            perf_mode=DR)
            if alt == 2:
                # qA: evacuate cols 0:256 first — exactly what the
                # first two score blocks need as lhsT — then the rest.
                for oc in range(OC):
                    evac(t_sb[:, oc, 0:2 * P], pt[:, oc, 0:2 * P],
                         bias0 + oc, "dve" if oc == 0 else "act")
                for oc in range(OC):
                    evac(t_sb[:, oc, 2 * P:2 * CCOL], pt[:, oc, 2 * P:],
                         bias0 + oc, "dve" if oc == 0 else "act")
                return
            for oc in range(OC):
                evac(t_sb[:, oc, half * 2 * CCOL:(half + 1) * 2 * CCOL],
                     pt[:, oc, :], bias0 + oc,
                     "dve" if (oc + alt) % 2 == 0 else "act")

        # --- projections, all fp8 DoubleRow (K=256 per instruction).
        kA = psS.tile([P, OC, 2 * CCOL], F32, tag="S", name="kA")
        proj(kA, wk_sb, kT_sb, 0, OC, 0)
        kB = psS.tile([P, OC, 2 * CCOL], F32, tag="S", name="kB")
        proj(kB, wk_sb, kT_sb, 1, OC, 1)
        # qT — ci=2,3 FIRST: then block 0's S tile takes over qB's PSUM
        # slot, which frees as soon as qB evacuates; qA (emitted last)
        # evacuates its first 256 columns early, which is all the data
        # blocks 0-1 need as lhsT.  (Keeping both q tiles out of the
        # S rotation also avoids mid-loop PSUM stalls.)
        qB = psS.tile([P, OC, 2 * CCOL], F32, tag="S", name="qB")
        proj(qB, wq_sb, qT_sb, 1, 0, 1)
        qA = psS.tile([P, OC, 2 * CCOL], F32, tag="S", name="qA")
        proj(qA, wq_sb, qT_sb, 0, 0, 2)

        Es = [None] * NBLK

        def stage1(b):
            # scores S = q @ k.T, 4 fp8 DoubleRow matmuls (K=256 each)
            S = psS.tile([P, N], F32, tag="S", name=f"S{b}")
            for ci in range(NCCOL):
                nc.tensor.matmul(
                    S[:, ci * CCOL:(ci + 1) * CCOL],
                    lhsT=qT_sb[:, :, b * P:(b + 1) * P],
                    rhs=kT_sb[:, :, ci * CCOL:(ci + 1) * CCOL],
                    start=True, stop=True, perf_mode=DR)
            # gate-multiply the band (host-prescaled by 1/Gv19)
            sb = _win_start(b)
            nc.vector.tensor_tensor(out=S[:, sb:sb + GW],
                                    in0=S[:, sb:sb + GW],
                                    in1=gb_sb[:, b * GW:b * GW + GW],
                                    op=Alu.mult)
            # whole-row exp, Z from the ACT accumulator
            E = epool.tile([P, N], BF16, tag="E")
            nc.scalar.activation(E[:], S[:], Act.Exp,
                                 scale=gv19_sb[:],
                                 accum_out=zall_sb[:, b:b + 1])
            Es[b] = E

        def stage2(b):
            # W1[q] = sum_m E[q,m]*vw[m] on DVE.  For the last two
            # blocks the DVE is the serial tail, so half the columns
            # move to Pool (product) + ACT (copy-accumulate), both of
            # which are idle after the final exp.
            E = Es[b]
            if b >= NBLK - 2:
                scp = sp_pool.tile([P, POOL_COLS], BF16, tag="scp")
                nc.gpsimd.tensor_tensor(out=scp[:], in0=E[:, :POOL_COLS],
                                        in1=vb_sb[:, :POOL_COLS],
                                        op=Alu.mult)
                nc.scalar.activation(scp[:], scp[:], Act.Identity,
                                     accum_out=w1p_sb[:, b:b + 1])
                scd = sd_pool.tile([P, N], BF16, tag="scd")
                nc.vector.scalar_tensor_tensor(
                    out=scd[:, POOL_COLS:], in0=E[:, POOL_COLS:],
                    scalar=1.0, in1=vb_sb[:, POOL_COLS:],
                    op0=Alu.bypass, op1=Alu.mult,
                    accum_out=w1d_sb[:, b:b + 1])
                return
            scd = sd_pool.tile([P, N], BF16, tag="scd")
            nc.vector.scalar_tensor_tensor(
                out=scd[:], in0=E[:], scalar=1.0, in1=vb_sb[:],
                op0=Alu.bypass, op1=Alu.mult,
                accum_out=w1d_sb[:, b:b + 1])

        def finish(lo, hi):
            # winner = 1 / (1 + exp(-(w1/Z + bs))), batched over blocks
            s = slice(lo, hi)
            if hi == NBLK:   # fold in the Pool/ACT partials of b=14,15
                nc.vector.tensor_tensor(
                    out=w1d_sb[:, NBLK - 2:], in0=w1d_sb[:, NBLK - 2:],
                    in1=w1p_sb[:, NBLK - 2:], op=Alu.add)
            izr = spool.tile([P, hi - lo], F32, tag="izr", name=f"izr{lo}")
            nc.vector.reciprocal(izr[:], zall_sb[:, s])
            w2 = spool.tile([P, hi - lo], F32, tag="w2", name=f"w2{lo}")
            nc.vector.tensor_tensor(out=w2[:], in0=w1d_sb[:, s], in1=izr[:],
                                    op=Alu.mult)
            we = spool.tile([P, hi - lo], F32, tag="we", name=f"we{lo}")
            nc.scalar.activation(we[:], w2[:], Act.Exp, bias=nbs_sb[:],
                                 scale=-1.0)
            wd = spool.tile([P, hi - lo], F32, tag="wd", name=f"wd{lo}")
            nc.vector.tensor_scalar_add(wd[:], we[:], 1.0)
            nc.vector.reciprocal(wout_sb[:, s], wd[:])
            nc.sync.dma_start(out[:, s], wout_sb[:, s])

        # --- main loop, software-pipelined: stage1(b+1) is emitted
        # before stage2(b) so the next block's gate+exp lead the queue.
        # qT ci=2,3 (first needed at block 8) is slotted in after
        # stage2(1): its PSUM buffer follows S1's slot cleanly and its
        # matmuls/evacs fill engine gaps during blocks 2-3.
        stage1(0)
        for b in range(NBLK):
            if b + 1 < NBLK:
                stage1(b + 1)
            stage2(b)
            if b == 8:
                finish(0, 8)
        finish(8, NBLK)

    nc.compile()
    return nc


def _gate_table(rank_emb, rank_w):
    d = np.arange(N)
    bucket = np.minimum(d // 5, NUM_BUCKETS - 1)
    emb = np.asarray(rank_emb, dtype=np.float64).reshape(-1)
    w = float(np.asarray(rank_w).reshape(-1)[0])
    gate = 1.0 / (1.0 + np.exp(-w * emb[bucket]))
    return np.ascontiguousarray((gate / np.sqrt(float(DOUT))).astype(np.float64))


_NC_CACHE = {}


def _get_nc(bs_val: float):
    key = float(np.float32(bs_val))
    if key not in _NC_CACHE:
        nc = bacc.Bacc("TRN2", target_bir_lowering=False, debug=False,
                       enable_asserts=False, num_devices=B)
        _NC_CACHE[key] = _build(nc, key)
    return _NC_CACHE[key]


def make_in_maps(inputs, bvs_host):
    import ml_dtypes
    BF = ml_dtypes.bfloat16
    E4 = ml_dtypes.float8_e4m3
    x = np.asarray(inputs["x"], dtype=np.float32)
    pr = np.asarray(inputs["price_rank"]).astype(np.int64)

    def _packw(w):
        # W.T [DIN, DOUT] * WG -> [P, DIN//P, DOUT] partition-major fp8
        wt = (np.asarray(w, np.float32).T * WG).astype(E4)
        return np.ascontiguousarray(
            wt.reshape(2 * NPAIR, P, DOUT).transpose(1, 0, 2)
            .reshape(P, 2 * NPAIR * DOUT))
    wq8 = _packw(inputs["Wq"])
    wk8 = _packw(inputs["Wk"])
    bq = np.asarray(inputs["bq"], np.float32) * SGAIN
    bk = np.asarray(inputs["bk"], np.float32) * SGAIN
    bqk = np.ascontiguousarray(
        np.stack([bq[:P], bq[P:], bk[:P], bk[P:]], axis=1))
    ws_vec = np.asarray(inputs["Ws"], np.float32).reshape(DOUT)
    wvs64 = (np.asarray(inputs["Wv"], np.float64).T
             @ ws_vec.astype(np.float64))
    gvt = _gate_table(inputs["rank_emb"], inputs["rank_w"])
    gv19_val = float(gvt[95])

    in_maps = []
    sigs = []
    for b in range(B):
        sig = np.argsort(pr[b], kind="stable")
        sigs.append(sig)
        xs = x[b][sig]
        prs = pr[b][sig]
        gl = np.empty((P, NBLK * GW), dtype=BF)
        for blk in range(NBLK):
            sb = _win_start(blk)
            rows = prs[blk * P:(blk + 1) * P]
            # outside the window the gate must equal the constant Gv19
            if sb > 0:
                assert rows.min() - prs[sb - 1] >= 95, (blk, "left")
            if sb + GW < N:
                assert prs[sb + GW] - rows.max() >= 95, (blk, "right")
            g = gvt[np.abs(rows[:, None] - prs[None, sb:sb + GW])] / gv19_val
            gl[:, blk * GW:(blk + 1) * GW] = g.astype(BF)
        vw = (xs.astype(np.float64) @ wvs64 + bvs_host).astype(np.float32)
        xp = np.ascontiguousarray(
            (xs.T * XG).astype(E4).reshape(2 * NPAIR, P, N)
            .transpose(1, 0, 2).reshape(P, 2 * NPAIR * N))
        in_maps.append({
            "xp": xp,
            "wq8": wq8, "wk8": wk8,
            "bqk": bqk,
            "gband": gl,
            "vb": np.ascontiguousarray(
                np.broadcast_to(vw.astype(BF).reshape(1, N), (P, N))),
            "gv19": np.full((P, 1), gv19_val / (SGAIN * SGAIN), dtype=np.float32),
        })
    return in_maps, sigs


def kernel(**inputs):
    global LAST_EXEC_NS
    bs_val = float(np.asarray(inputs["bs"]).reshape(-1)[0])
    ws_vec = np.asarray(inputs["Ws"], np.float64).reshape(DOUT)
    bvs_val = float(np.asarray(inputs["bv"], np.float64).reshape(DOUT) @ ws_vec)
    nc = _get_nc(bs_val)
    in_maps, sigs = make_in_maps(inputs, bvs_val)
    res = run_bass_kernel_spmd(nc, in_maps, list(range(B)))
    LAST_EXEC_NS = res.exec_time_ns
    globals()["LAST_RESULTS"] = res
    out = np.empty((B, N), dtype=np.float32)
    for b in range(B):
        ws = np.asarray(res.results[b]["out"], dtype=np.float32)  # [P, NBLK]
        out[b, sigs[b]] = ws.T.reshape(N)
    return out
